# revision 6
# baseline (speedup 1.0000x reference)
"""Trainium2 Bass kernel for multi-head attention (B=4, N=4096, D=384, H=6).

Sharding: 8 cores = 4 batches x 2 head-groups (3 heads each).
Each core computes, for its (batch, head-group):
    qkv = x @ W_g^T            (W_g = this group's q/k/v rows of W_qkv)
    per head: scoresT = (k q^T)^T laid out [keys(m) partition, queries(n) free]
    e = exp(scoresT * scale)   (no max subtraction; scores ~ N(0,1) after scale)
    PV with an appended ones-column in V: row 64 of the PSUM accumulator is
    the softmax denominator S[n].
    normalized headsT -> partial projection yT_g = Wp_g^T-contracted output
Host sums the two head-group partials per batch, transposes, adds bias.
"""

import os
import sys

import numpy as np

B, NSEQ, D = 4, 4096, 384
H, DH = 6, 64
HPC = 3  # heads per core
NCORES = 8
GCOLS = HPC * DH  # 192
SCALE = float(DH) ** -0.5


def _ensure_path():
    p = "/opt/trn_rl_repo"
    if os.path.isdir(p) and p not in sys.path:
        sys.path.insert(0, p)


def build_nc(n_seq=NSEQ):
    """Build the single-core Bass program (SPMD across 8 cores)."""
    _ensure_path()
    from contextlib import ExitStack

    import concourse.bacc as bacc
    import concourse.tile as tile
    from concourse import mybir
    from concourse.masks import make_identity

    f32 = mybir.dt.float32
    f32r = mybir.dt.float32r
    EXP = mybir.ActivationFunctionType.Exp

    assert n_seq % 512 == 0
    nt = n_seq // 128
    nch = n_seq // 512

    nc = bacc.Bacc("TRN2", target_bir_lowering=False, debug=False)

    x_d = nc.dram_tensor("x", [n_seq, D], f32, kind="ExternalInput")
    w_d = nc.dram_tensor("w_qkv", [3 * GCOLS, D], f32, kind="ExternalInput")
    wp_d = nc.dram_tensor("w_proj", [D, GCOLS], f32, kind="ExternalInput")
    y_d = nc.dram_tensor("yT", [D, n_seq], f32, kind="ExternalOutput")

    with tile.TileContext(nc) as tc, ExitStack() as ctx:
        constp = ctx.enter_context(tc.tile_pool(name="const", bufs=1))
        persist = ctx.enter_context(tc.tile_pool(name="persist", bufs=1))
        hpool = ctx.enter_context(tc.tile_pool(name="headsp", bufs=2))
        ypool = ctx.enter_context(tc.tile_pool(name="youtp", bufs=3))
        epool = ctx.enter_context(tc.tile_pool(name="epool", bufs=3))
        rcpool = ctx.enter_context(tc.tile_pool(name="rcp", bufs=2))

        ident = constp.tile([128, 128], f32)
        make_identity(nc, ident)
        ones = constp.tile([65, 64], f32)
        nc.vector.memset(ones, 1.0)

        # Transposed weights: wT[p, d3, e] = W_g[e, 128*d3 + p]
        wT = persist.tile([128, 3, 3 * GCOLS], f32r)
        # wpT[p, h, e] = Wp_g[e, 64*h + p]
        wpT = persist.tile([64, 3, D], f32r)
        # Per-head q/k in transposed layout [c, n]; pairs share base partition.
        t_q01 = persist.tile([128, n_seq], f32r)  # rows 0-63 qT_h0, 64-127 qT_h1
        t_k01 = persist.tile([128, n_seq], f32r)  # rows 0-63 kT_h0, 64-127 kT_h1
        t_q2 = persist.tile([64, n_seq], f32r)
        t_k2 = persist.tile([64, n_seq], f32r)
        # V in natural layout per n-tile, per head, with a ones column (col 64)
        v_sb = persist.tile([128, nt, 3, 65], f32r)
        ones_nt = constp.tile([128, nt * 3], f32)
        nc.vector.memset(ones_nt, 1.0)
        nc.vector.tensor_copy(
            v_sb[:, :, :, 64:65].rearrange("p t h o -> p (t h o)"), ones_nt
        )

        with (
            tc.tile_pool(name="ldw", bufs=1) as ldw,
            tc.tile_pool(name="ldx", bufs=3) as ldx,
            tc.tile_pool(name="xTp", bufs=1) as xTp,
            tc.tile_pool(name="ptr", bufs=2, space="PSUM") as ptr,
            tc.tile_pool(name="pqk", bufs=2, space="PSUM") as pqk,
            tc.tile_pool(name="pvp", bufs=2, space="PSUM") as pvp,
        ):
            # xT[p, d3, n] = x[n, 128*d3 + p]
            xT = xTp.tile([128, 3, n_seq], f32r)

            # ---- load + transpose weights ----
            for i in range(5):  # 576 rows -> 5 tiles (last is 64)
                rows = min(128, 3 * GCOLS - 128 * i)
                wn = ldw.tile([128, D], f32, tag="wn")
                nc.sync.dma_start(out=wn[:rows, :], in_=w_d[128 * i : 128 * i + rows, :])
                for d3 in range(3):
                    pt = ptr.tile([128, 128], f32, tag="pt")
                    nc.tensor.transpose(
                        pt[:, :rows],
                        wn[:rows, 128 * d3 : 128 * (d3 + 1)],
                        ident[:rows, :rows],
                    )
                    nc.vector.tensor_copy(wT[:, d3, 128 * i : 128 * i + rows], pt[:, :rows])

            for e3 in range(3):
                wpn = ldw.tile([128, GCOLS], f32, tag="wpn")
                nc.sync.dma_start(out=wpn, in_=wp_d[128 * e3 : 128 * (e3 + 1), :])
                for h in range(3):
                    pt = ptr.tile([128, 128], f32, tag="pt")
                    nc.tensor.transpose(pt[:64, :], wpn[:, 64 * h : 64 * (h + 1)], ident)
                    nc.vector.tensor_copy(wpT[:, h, 128 * e3 : 128 * (e3 + 1)], pt[:64, :])

            # ---- load + transpose x, then qkv per chunk ----
            for j in range(nch):
                for tt in range(4 * j, 4 * j + 4):
                    xn = ldx.tile([128, D], f32, tag="xn")
                    nc.sync.dma_start(out=xn, in_=x_d[128 * tt : 128 * (tt + 1), :])
                    for d3 in range(3):
                        pt = ptr.tile([128, 128], f32, tag="pt")
                        nc.tensor.transpose(pt, xn[:, 128 * d3 : 128 * (d3 + 1)], ident)
                        nc.vector.tensor_copy(xT[:, d3, 128 * tt : 128 * (tt + 1)], pt)

                # q/k (transposed layout), grouped so rows land at final bases
                for c0, m, dest in (
                    (0, 128, t_q01),
                    (192, 128, t_k01),
                    (128, 64, t_q2),
                    (320, 64, t_k2),
                ):
                    ps = pqk.tile([128, 512], f32, tag="pqk")
                    for d3 in range(3):
                        nc.tensor.matmul(
                            ps[:m, :],
                            wT[:, d3, c0 : c0 + m],
                            xT[:, d3, 512 * j : 512 * (j + 1)],
                            start=(d3 == 0),
                            stop=(d3 == 2),
                        )
                    nc.vector.tensor_copy(dest[:m, 512 * j : 512 * (j + 1)], ps[:m, :])

                # v in natural layout (rhs widened to 256 cols for full PE rate)
                for tt in range(4 * j, 4 * j + 4):
                    ps = pvp.tile([128, 256], f32, tag="pv")
                    for d3 in range(3):
                        nc.tensor.matmul(
                            ps,
                            xT[:, d3, 128 * tt : 128 * (tt + 1)],
                            wT[:, d3, 320:576],
                            start=(d3 == 0),
                            stop=(d3 == 2),
                        )
                    nc.vector.tensor_copy(
                        v_sb[:, tt, :, 0:64],
                        ps[:, 64:256].rearrange("p (h c) -> p h c", h=3),
                    )

        # ---- attention + projection ----
        heads_qk = [(t_k01, t_q01, 0), (t_k01, t_q01, 64), (t_k2, t_q2, 0)]
        with (
            tc.tile_pool(name="psc", bufs=2, space="PSUM") as psc,
            tc.tile_pool(name="pop", bufs=2, space="PSUM") as pop,
            tc.tile_pool(name="pbp", bufs=1, space="PSUM") as pbp,
            tc.tile_pool(name="ppp", bufs=3, space="PSUM") as ppp,
        ):
            for j in range(nch):
                headsT = hpool.tile([64, 3, 512], f32r, tag="headsT")
                for h in range(3):
                    kt, qt, pb = heads_qk[h]
                    po_t = pop.tile([65, 512], f32, tag="po")
                    for t in range(nt):
                        ps = psc.tile([128, 512], f32, tag="ps")
                        nc.tensor.matmul(
                            ps,
                            kt[pb : pb + 64, 128 * t : 128 * (t + 1)],
                            qt[pb : pb + 64, 512 * j : 512 * (j + 1)],
                        )
                        e_t = epool.tile([128, 512], f32r, tag="e_t")
                        nc.scalar.activation(e_t, ps, EXP, scale=SCALE)
                        nc.tensor.matmul(
                            po_t,
                            v_sb[:, t, h, :],
                            e_t[:],
                            start=(t == 0),
                            stop=(t == nt - 1),
                        )
                    rc = rcpool.tile([65, 512], f32, tag="rc")
                    nc.vector.reciprocal(rc[64:65, :], po_t[64:65, :])
                    # broadcast 1/S across 64 partitions via K=1 fp32 matmul
                    pb_t = pbp.tile([64, 512], f32, tag="pb")
                    nc.tensor.matmul(pb_t, ones[64:65, :], rc[64:65, :])
                    rb = rcpool.tile([64, 512], f32, tag="rb")
                    nc.vector.tensor_copy(rb, pb_t)
                    nc.vector.tensor_mul(headsT[:, h, :], po_t[0:64, :], rb)

                for e3 in range(3):
                    pp_t = ppp.tile([128, 512], f32, tag="pp")
                    for h in range(3):
                        nc.tensor.matmul(
                            pp_t,
                            wpT[:, h, 128 * e3 : 128 * (e3 + 1)],
                            headsT[:, h, :],
                            start=(h == 0),
                            stop=(h == 2),
                        )
                    y_sb = ypool.tile([128, 512], f32, tag="y_sb")
                    nc.vector.tensor_copy(y_sb, pp_t)
                    nc.sync.dma_start(
                        out=y_d[128 * e3 : 128 * (e3 + 1), 512 * j : 512 * (j + 1)],
                        in_=y_sb,
                    )

    nc.compile()
    return nc


def shard_inputs(x, W_qkv, W_proj):
    """Full inputs -> per-core input maps."""
    x = np.ascontiguousarray(np.asarray(x, dtype=np.float32))
    W_qkv = np.ascontiguousarray(np.asarray(W_qkv, dtype=np.float32))
    W_proj = np.ascontiguousarray(np.asarray(W_proj, dtype=np.float32))
    d = W_qkv.shape[1]
    in_maps = []
    for c in range(NCORES):
        b, g = divmod(c, 2)
        sl = slice(GCOLS * g, GCOLS * (g + 1))
        w_g = np.concatenate([W_qkv[0 * d :][sl], W_qkv[1 * d :][sl], W_qkv[2 * d :][sl]], axis=0)
        wp_g = W_proj[:, sl]
        in_maps.append(
            {
                "x": np.ascontiguousarray(x[b]),
                "w_qkv": np.ascontiguousarray(w_g),
                "w_proj": np.ascontiguousarray(wp_g),
            }
        )
    return in_maps


def combine_outputs(yTs, b_proj, n_seq=NSEQ):
    """Per-core yT partials -> full [B, N, D] output."""
    b_proj = np.asarray(b_proj, dtype=np.float32)
    y = np.empty((B, n_seq, D), dtype=np.float32)
    for b in range(B):
        y[b] = (yTs[2 * b] + yTs[2 * b + 1]).T + b_proj
    return y


_NC_CACHE = {}


def kernel(**inputs):
    _ensure_path()
    from concourse.bass_utils import run_bass_kernel_spmd

    x = np.asarray(inputs["x"], dtype=np.float32)
    W_qkv = np.asarray(inputs["W_qkv"], dtype=np.float32)
    W_proj = np.asarray(inputs["W_proj"], dtype=np.float32)
    b_proj = np.asarray(inputs["b_proj"], dtype=np.float32)

    n_seq = x.shape[1]
    if n_seq not in _NC_CACHE:
        _NC_CACHE[n_seq] = build_nc(n_seq)
    nc = _NC_CACHE[n_seq]

    in_maps = shard_inputs(x, W_qkv, W_proj)
    res = run_bass_kernel_spmd(nc, in_maps, core_ids=list(range(NCORES)))
    yTs = [r["yT"] for r in res.results]
    return combine_outputs(yTs, b_proj, n_seq)


if __name__ == "__main__":
    rng = np.random.default_rng(0)
    n = 512
    x = rng.standard_normal((B, n, D), dtype=np.float32)
    wq = (rng.standard_normal((3 * D, D), dtype=np.float32) / np.sqrt(D)).astype(np.float32)
    wp = (rng.standard_normal((D, D), dtype=np.float32) / np.sqrt(D)).astype(np.float32)
    bp = np.zeros(D, np.float32)
    out = kernel(x=x, W_qkv=wq, W_proj=wp, b_proj=bp)
    print(out.shape, out.dtype)


# revision 8
# speedup vs baseline: 1.0093x; 1.0093x over previous
"""Trainium2 Bass kernel for multi-head attention (B=4, N=4096, D=384, H=6).

Sharding: 8 cores = 4 batches x 2 head-groups (3 heads each).
Each core computes, for its (batch, head-group):
    qkv = x @ W_g^T            (W_g = this group's q/k/v rows of W_qkv)
    per head: scoresT = (k q^T)^T laid out [keys(m) partition, queries(n) free]
    e = exp(scoresT * scale)   (no max subtraction; scores ~ N(0,1) after scale)
    PV with an appended ones-column in V: row 64 of the PSUM accumulator is
    the softmax denominator S[n].
    normalized headsT -> partial projection yT_g = Wp_g^T-contracted output
Host sums the two head-group partials per batch, transposes, adds bias.
"""

import os
import sys

import numpy as np

B, NSEQ, D = 4, 4096, 384
H, DH = 6, 64
HPC = 3  # heads per core
NCORES = 8
GCOLS = HPC * DH  # 192
SCALE = float(DH) ** -0.5


def _ensure_path():
    p = "/opt/trn_rl_repo"
    if os.path.isdir(p) and p not in sys.path:
        sys.path.insert(0, p)


def build_nc(n_seq=NSEQ):
    """Build the single-core Bass program (SPMD across 8 cores)."""
    _ensure_path()
    from contextlib import ExitStack

    import concourse.bacc as bacc
    import concourse.tile as tile
    from concourse import mybir
    from concourse.masks import make_identity

    f32 = mybir.dt.float32
    f32r = mybir.dt.float32r
    EXP = mybir.ActivationFunctionType.Exp

    assert n_seq % 512 == 0
    nt = n_seq // 128
    nch = n_seq // 512

    nc = bacc.Bacc("TRN2", target_bir_lowering=False, debug=False)

    x_d = nc.dram_tensor("x", [n_seq, D], f32, kind="ExternalInput")
    w_d = nc.dram_tensor("w_qkv", [3 * GCOLS, D], f32, kind="ExternalInput")
    wp_d = nc.dram_tensor("w_proj", [D, GCOLS], f32, kind="ExternalInput")
    y_d = nc.dram_tensor("yT", [D, n_seq], f32, kind="ExternalOutput")

    with tile.TileContext(nc) as tc, ExitStack() as ctx:
        constp = ctx.enter_context(tc.tile_pool(name="const", bufs=1))
        persist = ctx.enter_context(tc.tile_pool(name="persist", bufs=1))
        hpool = ctx.enter_context(tc.tile_pool(name="headsp", bufs=2))
        ypool = ctx.enter_context(tc.tile_pool(name="youtp", bufs=3))
        epool = ctx.enter_context(tc.tile_pool(name="epool", bufs=3))
        rcpool = ctx.enter_context(tc.tile_pool(name="rcp", bufs=2))

        ident = constp.tile([128, 128], f32)
        make_identity(nc, ident)
        ones = constp.tile([65, 64], f32)
        nc.vector.memset(ones, 1.0)

        # Transposed weights: wT[p, d3, e] = W_g[e, 128*d3 + p]
        wT = persist.tile([128, 3, 3 * GCOLS], f32r)
        # wpT[p, h, e] = Wp_g[e, 64*h + p]
        wpT = persist.tile([64, 3, D], f32r)
        # Per-head q/k in transposed layout [c, n]; pairs share base partition.
        t_q01 = persist.tile([128, n_seq], f32r)  # rows 0-63 qT_h0, 64-127 qT_h1
        t_k01 = persist.tile([128, n_seq], f32r)  # rows 0-63 kT_h0, 64-127 kT_h1
        t_q2 = persist.tile([64, n_seq], f32r)
        t_k2 = persist.tile([64, n_seq], f32r)
        # V in natural layout per n-tile, per head, with a ones column (col 64)
        v_sb = persist.tile([128, nt, 3, 65], f32r)
        ones_nt = constp.tile([128, nt * 3], f32)
        nc.vector.memset(ones_nt, 1.0)
        nc.vector.tensor_copy(
            v_sb[:, :, :, 64:65].rearrange("p t h o -> p (t h o)"), ones_nt
        )

        with (
            tc.tile_pool(name="ldw", bufs=1) as ldw,
            tc.tile_pool(name="ldx", bufs=3) as ldx,
            tc.tile_pool(name="xTp", bufs=1) as xTp,
            tc.tile_pool(name="ptr", bufs=4, space="PSUM") as ptr,
            tc.tile_pool(name="pqk", bufs=2, space="PSUM") as pqk,
            tc.tile_pool(name="pvp", bufs=2, space="PSUM") as pvp,
        ):
            # xT[p, d3, n] = x[n, 128*d3 + p]
            xT = xTp.tile([128, 3, n_seq], f32r)

            # ---- load + transpose weights ----
            for i in range(5):  # 576 rows -> 5 tiles (last is 64)
                rows = min(128, 3 * GCOLS - 128 * i)
                wn = ldw.tile([128, D], f32, tag="wn")
                nc.sync.dma_start(out=wn[:rows, :], in_=w_d[128 * i : 128 * i + rows, :])
                for d3 in range(3):
                    pt = ptr.tile([128, 128], f32, tag="pt")
                    nc.tensor.transpose(
                        pt[:, :rows],
                        wn[:rows, 128 * d3 : 128 * (d3 + 1)],
                        ident[:rows, :rows],
                    )
                    nc.vector.tensor_copy(wT[:, d3, 128 * i : 128 * i + rows], pt[:, :rows])

            for e3 in range(3):
                wpn = ldw.tile([128, GCOLS], f32, tag="wpn")
                nc.sync.dma_start(out=wpn, in_=wp_d[128 * e3 : 128 * (e3 + 1), :])
                for h in range(3):
                    pt = ptr.tile([128, 128], f32, tag="pt")
                    nc.tensor.transpose(pt[:64, :], wpn[:, 64 * h : 64 * (h + 1)], ident)
                    nc.vector.tensor_copy(wpT[:, h, 128 * e3 : 128 * (e3 + 1)], pt[:64, :])

            # ---- load + transpose x, then qkv per chunk ----
            for j in range(nch):
                for tt in range(4 * j, 4 * j + 4):
                    xn = ldx.tile([128, D], f32, tag="xn")
                    nc.sync.dma_start(out=xn, in_=x_d[128 * tt : 128 * (tt + 1), :])
                    for d3 in range(3):
                        pt = ptr.tile([128, 128], f32, tag="pt")
                        nc.tensor.transpose(pt, xn[:, 128 * d3 : 128 * (d3 + 1)], ident)
                        nc.vector.tensor_copy(xT[:, d3, 128 * tt : 128 * (tt + 1)], pt)

                # q/k (transposed layout), grouped so rows land at final bases
                for c0, m, dest in (
                    (0, 128, t_q01),
                    (192, 128, t_k01),
                    (128, 64, t_q2),
                    (320, 64, t_k2),
                ):
                    ps = pqk.tile([128, 512], f32, tag="pqk")
                    for d3 in range(3):
                        nc.tensor.matmul(
                            ps[:m, :],
                            wT[:, d3, c0 : c0 + m],
                            xT[:, d3, 512 * j : 512 * (j + 1)],
                            start=(d3 == 0),
                            stop=(d3 == 2),
                        )
                    nc.vector.tensor_copy(dest[:m, 512 * j : 512 * (j + 1)], ps[:m, :])

                # v in natural layout (rhs widened to 256 cols for full PE rate)
                for tt in range(4 * j, 4 * j + 4):
                    ps = pvp.tile([128, 256], f32, tag="pv")
                    for d3 in range(3):
                        nc.tensor.matmul(
                            ps,
                            xT[:, d3, 128 * tt : 128 * (tt + 1)],
                            wT[:, d3, 320:576],
                            start=(d3 == 0),
                            stop=(d3 == 2),
                        )
                    nc.vector.tensor_copy(
                        v_sb[:, tt, :, 0:64],
                        ps[:, 64:256].rearrange("p (h c) -> p h c", h=3),
                    )

        # ---- attention + projection ----
        # Pipelined: scores for t-pairs into a 2-bank PSUM tile, one exp ACT
        # per pair, PV accumulation trailing. Each head's normalize +
        # projection tail is deferred into the next head's compute so the PE
        # never drains (keeps the HAM clock un-throttled).
        heads_qk = [(t_k01, t_q01, 0), (t_k01, t_q01, 64), (t_k2, t_q2, 0)]
        with (
            tc.tile_pool(name="psc", bufs=2, space="PSUM") as psc,
            tc.tile_pool(name="pop", bufs=2, space="PSUM") as pop,
            tc.tile_pool(name="misc", bufs=2, space="PSUM") as misc,
        ):

            def make_tail(j, h, po_t, headsT):
                def tail():
                    rc = rcpool.tile([65, 512], f32, tag="rc")
                    nc.vector.reciprocal(rc[64:65, :], po_t[64:65, :])
                    # broadcast 1/S across 64 partitions via K=1 fp32 matmul
                    pb_t = misc.tile([64, 512], f32, tag="misc")
                    nc.tensor.matmul(pb_t, ones[64:65, :], rc[64:65, :])
                    rb = rcpool.tile([64, 512], f32, tag="rb")
                    nc.vector.tensor_copy(rb, pb_t)
                    nc.vector.tensor_mul(headsT[:, h, :], po_t[0:64, :], rb)
                    if h == 2:
                        for e3 in range(3):
                            pp_t = misc.tile([128, 512], f32, tag="misc")
                            for hh in range(3):
                                nc.tensor.matmul(
                                    pp_t,
                                    wpT[:, hh, 128 * e3 : 128 * (e3 + 1)],
                                    headsT[:, hh, :],
                                    start=(hh == 0),
                                    stop=(hh == 2),
                                )
                            y_sb = ypool.tile([128, 512], f32, tag="y_sb")
                            nc.vector.tensor_copy(y_sb, pp_t)
                            nc.sync.dma_start(
                                out=y_d[
                                    128 * e3 : 128 * (e3 + 1),
                                    512 * j : 512 * (j + 1),
                                ],
                                in_=y_sb,
                            )

                return tail

            pending_tail = None
            for j in range(nch):
                headsT = hpool.tile([64, 3, 512], f32r, tag="headsT")
                for h in range(3):
                    kt, qt, pb = heads_qk[h]
                    po_t = pop.tile([65, 512], f32, tag="po")
                    for tp in range(nt // 2):
                        t0 = 2 * tp
                        ps = psc.tile([128, 2, 512], f32, tag="ps")
                        for s in range(2):
                            t = t0 + s
                            nc.tensor.matmul(
                                ps[:, s, :],
                                kt[pb : pb + 64, 128 * t : 128 * (t + 1)],
                                qt[pb : pb + 64, 512 * j : 512 * (j + 1)],
                            )
                        e_t = epool.tile([128, 2, 512], f32r, tag="e_t")
                        nc.scalar.activation(e_t, ps[:], EXP, scale=SCALE)
                        for s in range(2):
                            t = t0 + s
                            nc.tensor.matmul(
                                po_t,
                                v_sb[:, t, h, :],
                                e_t[:, s, :],
                                start=(t == 0),
                                stop=(t == nt - 1),
                            )
                        if tp == 1 and pending_tail is not None:
                            pending_tail()
                            pending_tail = None
                    pending_tail = make_tail(j, h, po_t, headsT)
            pending_tail()

    nc.compile()
    return nc


def shard_inputs(x, W_qkv, W_proj):
    """Full inputs -> per-core input maps."""
    x = np.ascontiguousarray(np.asarray(x, dtype=np.float32))
    W_qkv = np.ascontiguousarray(np.asarray(W_qkv, dtype=np.float32))
    W_proj = np.ascontiguousarray(np.asarray(W_proj, dtype=np.float32))
    d = W_qkv.shape[1]
    in_maps = []
    for c in range(NCORES):
        b, g = divmod(c, 2)
        sl = slice(GCOLS * g, GCOLS * (g + 1))
        w_g = np.concatenate([W_qkv[0 * d :][sl], W_qkv[1 * d :][sl], W_qkv[2 * d :][sl]], axis=0)
        wp_g = W_proj[:, sl]
        in_maps.append(
            {
                "x": np.ascontiguousarray(x[b]),
                "w_qkv": np.ascontiguousarray(w_g),
                "w_proj": np.ascontiguousarray(wp_g),
            }
        )
    return in_maps


def combine_outputs(yTs, b_proj, n_seq=NSEQ):
    """Per-core yT partials -> full [B, N, D] output."""
    b_proj = np.asarray(b_proj, dtype=np.float32)
    y = np.empty((B, n_seq, D), dtype=np.float32)
    for b in range(B):
        y[b] = (yTs[2 * b] + yTs[2 * b + 1]).T + b_proj
    return y


_NC_CACHE = {}


def kernel(**inputs):
    _ensure_path()
    from concourse.bass_utils import run_bass_kernel_spmd

    x = np.asarray(inputs["x"], dtype=np.float32)
    W_qkv = np.asarray(inputs["W_qkv"], dtype=np.float32)
    W_proj = np.asarray(inputs["W_proj"], dtype=np.float32)
    b_proj = np.asarray(inputs["b_proj"], dtype=np.float32)

    n_seq = x.shape[1]
    if n_seq not in _NC_CACHE:
        _NC_CACHE[n_seq] = build_nc(n_seq)
    nc = _NC_CACHE[n_seq]

    in_maps = shard_inputs(x, W_qkv, W_proj)
    res = run_bass_kernel_spmd(nc, in_maps, core_ids=list(range(NCORES)))
    yTs = [r["yT"] for r in res.results]
    return combine_outputs(yTs, b_proj, n_seq)


if __name__ == "__main__":
    rng = np.random.default_rng(0)
    n = 512
    x = rng.standard_normal((B, n, D), dtype=np.float32)
    wq = (rng.standard_normal((3 * D, D), dtype=np.float32) / np.sqrt(D)).astype(np.float32)
    wp = (rng.standard_normal((D, D), dtype=np.float32) / np.sqrt(D)).astype(np.float32)
    bp = np.zeros(D, np.float32)
    out = kernel(x=x, W_qkv=wq, W_proj=wp, b_proj=bp)
    print(out.shape, out.dtype)


# revision 10
# speedup vs baseline: 1.0201x; 1.0107x over previous
"""Trainium2 Bass kernel for multi-head attention (B=4, N=4096, D=384, H=6).

Sharding: 8 cores = 4 batches x 2 head-groups (3 heads each).
Each core computes, for its (batch, head-group):
    qkv = x @ W_g^T            (W_g = this group's q/k/v rows of W_qkv)
    per head: scoresT = (k q^T)^T laid out [keys(m) partition, queries(n) free]
    e = exp(scoresT * scale)   (no max subtraction; scores ~ N(0,1) after scale)
    PV with an appended ones-column in V: row 64 of the PSUM accumulator is
    the softmax denominator S[n].
    normalized headsT -> partial projection yT_g = Wp_g^T-contracted output
Host sums the two head-group partials per batch, transposes, adds bias.
"""

import os
import sys

import numpy as np

B, NSEQ, D = 4, 4096, 384
H, DH = 6, 64
HPC = 3  # heads per core
NCORES = 8
GCOLS = HPC * DH  # 192
SCALE = float(DH) ** -0.5


def _ensure_path():
    p = "/opt/trn_rl_repo"
    if os.path.isdir(p) and p not in sys.path:
        sys.path.insert(0, p)


def build_nc(n_seq=NSEQ):
    """Build the single-core Bass program (SPMD across 8 cores)."""
    _ensure_path()
    from contextlib import ExitStack

    import concourse.bacc as bacc
    import concourse.tile as tile
    from concourse import mybir
    from concourse.masks import make_identity

    f32 = mybir.dt.float32
    f32r = mybir.dt.float32r
    EXP = mybir.ActivationFunctionType.Exp

    assert n_seq % 512 == 0
    nt = n_seq // 128
    nch = n_seq // 512

    nc = bacc.Bacc("TRN2", target_bir_lowering=False, debug=False)

    x_d = nc.dram_tensor("x", [n_seq, D], f32, kind="ExternalInput")
    w_d = nc.dram_tensor("w_qkv", [3 * GCOLS, D], f32, kind="ExternalInput")
    wp_d = nc.dram_tensor("w_proj", [D, GCOLS], f32, kind="ExternalInput")
    y_d = nc.dram_tensor("yT", [D, n_seq], f32, kind="ExternalOutput")

    with tile.TileContext(nc) as tc, ExitStack() as ctx:
        constp = ctx.enter_context(tc.tile_pool(name="const", bufs=1))
        persist = ctx.enter_context(tc.tile_pool(name="persist", bufs=1))
        hpool = ctx.enter_context(tc.tile_pool(name="headsp", bufs=2))
        ypool = ctx.enter_context(tc.tile_pool(name="youtp", bufs=3))
        epool = ctx.enter_context(tc.tile_pool(name="epool", bufs=4))
        rcpool = ctx.enter_context(tc.tile_pool(name="rcp", bufs=2))

        ident = constp.tile([128, 128], f32)
        make_identity(nc, ident)
        ones = constp.tile([65, 64], f32)
        nc.vector.memset(ones, 1.0)

        # Transposed weights: wT[p, d3, e] = W_g[e, 128*d3 + p]
        wT = persist.tile([128, 3, 3 * GCOLS], f32r)
        # wpT[p, h, e] = Wp_g[e, 64*h + p]
        wpT = persist.tile([64, 3, D], f32r)
        # Per-head q/k in transposed layout [c, n]; pairs share base partition.
        t_q01 = persist.tile([128, n_seq], f32r)  # rows 0-63 qT_h0, 64-127 qT_h1
        t_k01 = persist.tile([128, n_seq], f32r)  # rows 0-63 kT_h0, 64-127 kT_h1
        t_q2 = persist.tile([64, n_seq], f32r)
        t_k2 = persist.tile([64, n_seq], f32r)
        # V in natural layout per n-tile, per head, with a ones column (col 64)
        v_sb = persist.tile([128, nt, 3, 65], f32r)
        ones_nt = constp.tile([128, nt * 3], f32)
        nc.vector.memset(ones_nt, 1.0)
        nc.vector.tensor_copy(
            v_sb[:, :, :, 64:65].rearrange("p t h o -> p (t h o)"), ones_nt
        )

        with (
            tc.tile_pool(name="ldw", bufs=1) as ldw,
            tc.tile_pool(name="ldx", bufs=3) as ldx,
            tc.tile_pool(name="xTp", bufs=1) as xTp,
            tc.tile_pool(name="ptr", bufs=4, space="PSUM") as ptr,
            tc.tile_pool(name="pqk", bufs=2, space="PSUM") as pqk,
            tc.tile_pool(name="pvp", bufs=2, space="PSUM") as pvp,
        ):
            # xT[p, d3, n] = x[n, 128*d3 + p]
            xT = xTp.tile([128, 3, n_seq], f32r)

            # ---- load + transpose weights ----
            for i in range(5):  # 576 rows -> 5 tiles (last is 64)
                rows = min(128, 3 * GCOLS - 128 * i)
                wn = ldw.tile([128, D], f32, tag="wn")
                nc.sync.dma_start(out=wn[:rows, :], in_=w_d[128 * i : 128 * i + rows, :])
                for d3 in range(3):
                    pt = ptr.tile([128, 128], f32, tag="pt")
                    nc.tensor.transpose(
                        pt[:, :rows],
                        wn[:rows, 128 * d3 : 128 * (d3 + 1)],
                        ident[:rows, :rows],
                    )
                    nc.vector.tensor_copy(wT[:, d3, 128 * i : 128 * i + rows], pt[:, :rows])

            for e3 in range(3):
                wpn = ldw.tile([128, GCOLS], f32, tag="wpn")
                nc.sync.dma_start(out=wpn, in_=wp_d[128 * e3 : 128 * (e3 + 1), :])
                for h in range(3):
                    pt = ptr.tile([128, 128], f32, tag="pt")
                    nc.tensor.transpose(pt[:64, :], wpn[:, 64 * h : 64 * (h + 1)], ident)
                    nc.vector.tensor_copy(wpT[:, h, 128 * e3 : 128 * (e3 + 1)], pt[:64, :])

            # ---- load + transpose x, then qkv per chunk ----
            for j in range(nch):
                for tt in range(4 * j, 4 * j + 4):
                    xn = ldx.tile([128, D], f32, tag="xn")
                    nc.sync.dma_start(out=xn, in_=x_d[128 * tt : 128 * (tt + 1), :])
                    for d3 in range(3):
                        pt = ptr.tile([128, 128], f32, tag="pt")
                        nc.tensor.transpose(pt, xn[:, 128 * d3 : 128 * (d3 + 1)], ident)
                        nc.vector.tensor_copy(xT[:, d3, 128 * tt : 128 * (tt + 1)], pt)

                # q/k (transposed layout), grouped so rows land at final bases
                for c0, m, dest in (
                    (0, 128, t_q01),
                    (192, 128, t_k01),
                    (128, 64, t_q2),
                    (320, 64, t_k2),
                ):
                    ps = pqk.tile([128, 512], f32, tag="pqk")
                    for d3 in range(3):
                        nc.tensor.matmul(
                            ps[:m, :],
                            wT[:, d3, c0 : c0 + m],
                            xT[:, d3, 512 * j : 512 * (j + 1)],
                            start=(d3 == 0),
                            stop=(d3 == 2),
                        )
                    nc.vector.tensor_copy(dest[:m, 512 * j : 512 * (j + 1)], ps[:m, :])

                # v in natural layout (rhs widened to 256 cols for full PE rate)
                for tt in range(4 * j, 4 * j + 4):
                    ps = pvp.tile([128, 256], f32, tag="pv")
                    for d3 in range(3):
                        nc.tensor.matmul(
                            ps,
                            xT[:, d3, 128 * tt : 128 * (tt + 1)],
                            wT[:, d3, 320:576],
                            start=(d3 == 0),
                            stop=(d3 == 2),
                        )
                    nc.vector.tensor_copy(
                        v_sb[:, tt, :, 0:64],
                        ps[:, 64:256].rearrange("p (h c) -> p h c", h=3),
                    )

        # ---- attention + projection ----
        # Pipelined: scores for t-pairs into a 2-bank PSUM tile, one exp ACT
        # per pair, PV accumulation trailing. Each head's normalize +
        # projection tail is deferred into the next head's compute so the PE
        # never drains (keeps the HAM clock un-throttled).
        heads_qk = [(t_k01, t_q01, 0), (t_k01, t_q01, 64), (t_k2, t_q2, 0)]
        with (
            tc.tile_pool(name="psc", bufs=2, space="PSUM") as psc,
            tc.tile_pool(name="pop", bufs=2, space="PSUM") as pop,
            tc.tile_pool(name="misc", bufs=2, space="PSUM") as misc,
        ):

            def make_tail(j, h, po_t, headsT):
                def tail():
                    rc = rcpool.tile([65, 512], f32, tag="rc")
                    nc.vector.reciprocal(rc[64:65, :], po_t[64:65, :])
                    # broadcast 1/S across 64 partitions via K=1 fp32 matmul
                    pb_t = misc.tile([64, 512], f32, tag="misc")
                    nc.tensor.matmul(pb_t, ones[64:65, :], rc[64:65, :])
                    rb = rcpool.tile([64, 512], f32, tag="rb")
                    nc.vector.tensor_copy(rb, pb_t)
                    nc.vector.tensor_mul(headsT[:, h, :], po_t[0:64, :], rb)
                    if h == 2:
                        for e3 in range(3):
                            pp_t = misc.tile([128, 512], f32, tag="misc")
                            for hh in range(3):
                                nc.tensor.matmul(
                                    pp_t,
                                    wpT[:, hh, 128 * e3 : 128 * (e3 + 1)],
                                    headsT[:, hh, :],
                                    start=(hh == 0),
                                    stop=(hh == 2),
                                )
                            y_sb = ypool.tile([128, 512], f32, tag="y_sb")
                            nc.vector.tensor_copy(y_sb, pp_t)
                            nc.sync.dma_start(
                                out=y_d[
                                    128 * e3 : 128 * (e3 + 1),
                                    512 * j : 512 * (j + 1),
                                ],
                                in_=y_sb,
                            )

                return tail

            PV_DELAY = 2  # pairs the PV matmuls trail the exp by

            pending_tail = None
            for j in range(nch):
                headsT = hpool.tile([64, 3, 512], f32r, tag="headsT")
                for h in range(3):
                    kt, qt, pb = heads_qk[h]
                    po_t = pop.tile([65, 512], f32, tag="po")
                    pv_queue = []

                    def flush_pv(po_t=po_t, h=h):
                        t0, e_t = pv_queue.pop(0)
                        for s in range(2):
                            t = t0 + s
                            nc.tensor.matmul(
                                po_t,
                                v_sb[:, t, h, :],
                                e_t[:, s, :],
                                start=(t == 0),
                                stop=(t == nt - 1),
                            )

                    for tp in range(nt // 2):
                        t0 = 2 * tp
                        ps = psc.tile([128, 2, 512], f32, tag="ps")
                        for s in range(2):
                            t = t0 + s
                            nc.tensor.matmul(
                                ps[:, s, :],
                                kt[pb : pb + 64, 128 * t : 128 * (t + 1)],
                                qt[pb : pb + 64, 512 * j : 512 * (j + 1)],
                            )
                        e_t = epool.tile([128, 2, 512], f32r, tag="e_t")
                        nc.scalar.activation(e_t, ps[:], EXP, scale=SCALE)
                        pv_queue.append((t0, e_t))
                        if len(pv_queue) > PV_DELAY:
                            flush_pv()
                        if tp == 1 and pending_tail is not None:
                            pending_tail()
                            pending_tail = None
                    while pv_queue:
                        flush_pv()
                    pending_tail = make_tail(j, h, po_t, headsT)
            pending_tail()

    nc.compile()
    return nc


def shard_inputs(x, W_qkv, W_proj):
    """Full inputs -> per-core input maps."""
    x = np.ascontiguousarray(np.asarray(x, dtype=np.float32))
    W_qkv = np.ascontiguousarray(np.asarray(W_qkv, dtype=np.float32))
    W_proj = np.ascontiguousarray(np.asarray(W_proj, dtype=np.float32))
    d = W_qkv.shape[1]
    in_maps = []
    for c in range(NCORES):
        b, g = divmod(c, 2)
        sl = slice(GCOLS * g, GCOLS * (g + 1))
        w_g = np.concatenate([W_qkv[0 * d :][sl], W_qkv[1 * d :][sl], W_qkv[2 * d :][sl]], axis=0)
        wp_g = W_proj[:, sl]
        in_maps.append(
            {
                "x": np.ascontiguousarray(x[b]),
                "w_qkv": np.ascontiguousarray(w_g),
                "w_proj": np.ascontiguousarray(wp_g),
            }
        )
    return in_maps


def combine_outputs(yTs, b_proj, n_seq=NSEQ):
    """Per-core yT partials -> full [B, N, D] output."""
    b_proj = np.asarray(b_proj, dtype=np.float32)
    y = np.empty((B, n_seq, D), dtype=np.float32)
    for b in range(B):
        y[b] = (yTs[2 * b] + yTs[2 * b + 1]).T + b_proj
    return y


_NC_CACHE = {}


def kernel(**inputs):
    _ensure_path()
    from concourse.bass_utils import run_bass_kernel_spmd

    x = np.asarray(inputs["x"], dtype=np.float32)
    W_qkv = np.asarray(inputs["W_qkv"], dtype=np.float32)
    W_proj = np.asarray(inputs["W_proj"], dtype=np.float32)
    b_proj = np.asarray(inputs["b_proj"], dtype=np.float32)

    n_seq = x.shape[1]
    if n_seq not in _NC_CACHE:
        _NC_CACHE[n_seq] = build_nc(n_seq)
    nc = _NC_CACHE[n_seq]

    in_maps = shard_inputs(x, W_qkv, W_proj)
    res = run_bass_kernel_spmd(nc, in_maps, core_ids=list(range(NCORES)))
    yTs = [r["yT"] for r in res.results]
    return combine_outputs(yTs, b_proj, n_seq)


if __name__ == "__main__":
    rng = np.random.default_rng(0)
    n = 512
    x = rng.standard_normal((B, n, D), dtype=np.float32)
    wq = (rng.standard_normal((3 * D, D), dtype=np.float32) / np.sqrt(D)).astype(np.float32)
    wp = (rng.standard_normal((D, D), dtype=np.float32) / np.sqrt(D)).astype(np.float32)
    bp = np.zeros(D, np.float32)
    out = kernel(x=x, W_qkv=wq, W_proj=wp, b_proj=bp)
    print(out.shape, out.dtype)


# revision 18
# speedup vs baseline: 1.9316x; 1.8934x over previous
"""Trainium2 Bass kernel for multi-head attention (B=4, N=4096, D=384, H=6).

Sharding: 8 cores = 4 batches x 2 head-groups (3 heads each).
Each core computes, for its (batch, head-group):
    qkv = x @ W_g^T            (W_g = this group's q/k/v rows of W_qkv)
    per head: scoresT = (k q^T)^T laid out [keys(m) partition, queries(n) free]
    e = exp(scoresT * scale)   (no max subtraction; scores ~ N(0,1) after scale)
    PV with an appended ones-column in V: row 64 of the PSUM accumulator is
    the softmax denominator S[n].
    normalized headsT -> partial projection yT_g = Wp_g^T-contracted output
Host sums the two head-group partials per batch, transposes, adds bias.
"""

import os
import sys

import numpy as np

B, NSEQ, D = 4, 4096, 384
H, DH = 6, 64
HPC = 3  # heads per core
NCORES = 8
GCOLS = HPC * DH  # 192
SCALE = float(DH) ** -0.5


def _ensure_path():
    p = "/opt/trn_rl_repo"
    if os.path.isdir(p) and p not in sys.path:
        sys.path.insert(0, p)


def build_nc(n_seq=NSEQ):
    """Build the single-core Bass program (SPMD across 8 cores)."""
    _ensure_path()
    from contextlib import ExitStack

    import concourse.bacc as bacc
    import concourse.tile as tile
    from concourse import mybir
    from concourse.masks import make_identity

    f32 = mybir.dt.float32
    f32r = mybir.dt.float32r
    bf16 = mybir.dt.bfloat16
    EXP = mybir.ActivationFunctionType.Exp

    assert n_seq % 512 == 0
    nt = n_seq // 128
    nch = n_seq // 512

    nc = bacc.Bacc("TRN2", target_bir_lowering=False, debug=False)

    x_d = nc.dram_tensor("x", [n_seq, D], f32, kind="ExternalInput")
    w_d = nc.dram_tensor("w_qkv", [3 * GCOLS, D], f32, kind="ExternalInput")
    wp_d = nc.dram_tensor("w_proj", [D, GCOLS], f32, kind="ExternalInput")
    y_d = nc.dram_tensor("yT", [D, n_seq], f32, kind="ExternalOutput")

    with tile.TileContext(nc) as tc, ExitStack() as ctx:
        constp = ctx.enter_context(tc.tile_pool(name="const", bufs=1))
        persist = ctx.enter_context(tc.tile_pool(name="persist", bufs=1))
        hpool = ctx.enter_context(tc.tile_pool(name="headsp", bufs=2))
        ypool = ctx.enter_context(tc.tile_pool(name="youtp", bufs=3))
        epool = ctx.enter_context(tc.tile_pool(name="epool", bufs=5))
        rcpool = ctx.enter_context(tc.tile_pool(name="rcp", bufs=2))

        ident = constp.tile([128, 128], f32)
        make_identity(nc, ident)
        ones = constp.tile([65, 64], f32)
        nc.vector.memset(ones, 1.0)

        # Transposed weights: wT[p, d3, e] = W_g[e, 128*d3 + p]
        wT = persist.tile([128, 3, 3 * GCOLS], f32r)
        # wpT[p, h, e] = Wp_g[e, 64*h + p]
        wpT = persist.tile([64, 3, D], bf16)
        # Per-head q/k in transposed layout [c, n], bf16, REPLICATED across
        # both 64-partition halves so score matmuls can row-pack in pairs.
        t_q0 = persist.tile([128, n_seq], bf16)
        t_q1 = persist.tile([128, n_seq], bf16)
        t_q2 = persist.tile([128, n_seq], bf16)
        t_k0 = persist.tile([128, n_seq], bf16)
        t_k1 = persist.tile([128, n_seq], bf16)
        t_k2 = persist.tile([128, n_seq], bf16)
        # V in natural layout per n-tile, per head, with a ones column (col 64)
        v_sb = persist.tile([128, nt, 3, 65], bf16)
        ones_nt = constp.tile([128, nt * 3], f32)
        nc.vector.memset(ones_nt, 1.0)
        nc.vector.tensor_copy(
            v_sb[:, :, :, 64:65].rearrange("p t h o -> p (t h o)"), ones_nt
        )

        with (
            tc.tile_pool(name="ldw", bufs=1) as ldw,
            tc.tile_pool(name="ldx", bufs=3) as ldx,
            tc.tile_pool(name="xTp", bufs=1) as xTp,
            tc.tile_pool(name="ptr", bufs=4, space="PSUM") as ptr,
            tc.tile_pool(name="pqk", bufs=2, space="PSUM") as pqk,
            tc.tile_pool(name="pvp", bufs=2, space="PSUM") as pvp,
        ):
            # xT[p, d3, n] = x[n, 128*d3 + p]
            xT = xTp.tile([128, 3, n_seq], f32r)

            # ---- load + transpose weights ----
            for i in range(5):  # 576 rows -> 5 tiles (last is 64)
                rows = min(128, 3 * GCOLS - 128 * i)
                wn = ldw.tile([128, D], f32, tag="wn")
                nc.sync.dma_start(out=wn[:rows, :], in_=w_d[128 * i : 128 * i + rows, :])
                for d3 in range(3):
                    pt = ptr.tile([128, 128], f32, tag="pt")
                    nc.tensor.transpose(
                        pt[:, :rows],
                        wn[:rows, 128 * d3 : 128 * (d3 + 1)],
                        ident[:rows, :rows],
                    )
                    nc.vector.tensor_copy(wT[:, d3, 128 * i : 128 * i + rows], pt[:, :rows])

            for e3 in range(3):
                wpn = ldw.tile([128, GCOLS], f32, tag="wpn")
                nc.sync.dma_start(out=wpn, in_=wp_d[128 * e3 : 128 * (e3 + 1), :])
                for h in range(3):
                    pt = ptr.tile([128, 128], f32, tag="pt")
                    nc.tensor.transpose(pt[:64, :], wpn[:, 64 * h : 64 * (h + 1)], ident)
                    nc.vector.tensor_copy(wpT[:, h, 128 * e3 : 128 * (e3 + 1)], pt[:64, :])

            # ---- load + transpose x, then qkv per chunk ----
            for j in range(nch):
                for tt in range(4 * j, 4 * j + 4):
                    xn = ldx.tile([128, D], f32, tag="xn")
                    nc.sync.dma_start(out=xn, in_=x_d[128 * tt : 128 * (tt + 1), :])
                    for d3 in range(3):
                        pt = ptr.tile([128, 128], f32, tag="pt")
                        nc.tensor.transpose(pt, xn[:, 128 * d3 : 128 * (d3 + 1)], ident)
                        nc.vector.tensor_copy(xT[:, d3, 128 * tt : 128 * (tt + 1)], pt)

                # q/k (transposed layout); each 64-row group is copied to the
                # matching partition half of its replicated destination tile.
                for c0, m, dests in (
                    (0, 128, ((t_q0, 0), (t_q1, 64))),
                    (192, 128, ((t_k0, 0), (t_k1, 64))),
                    (128, 64, ((t_q2, 0),)),
                    (320, 64, ((t_k2, 0),)),
                ):
                    ps = pqk.tile([128, 512], f32, tag="pqk")
                    for d3 in range(3):
                        nc.tensor.matmul(
                            ps[:m, :],
                            wT[:, d3, c0 : c0 + m],
                            xT[:, d3, 512 * j : 512 * (j + 1)],
                            start=(d3 == 0),
                            stop=(d3 == 2),
                        )
                    for dest, p0 in dests:
                        nc.vector.tensor_copy(
                            dest[p0 : p0 + 64, 512 * j : 512 * (j + 1)],
                            ps[p0 : p0 + 64, :],
                        )

                # v in natural layout (rhs widened to 256 cols for full PE rate)
                for tt in range(4 * j, 4 * j + 4):
                    ps = pvp.tile([128, 256], f32, tag="pv")
                    for d3 in range(3):
                        nc.tensor.matmul(
                            ps,
                            xT[:, d3, 128 * tt : 128 * (tt + 1)],
                            wT[:, d3, 320:576],
                            start=(d3 == 0),
                            stop=(d3 == 2),
                        )
                    nc.vector.tensor_copy(
                        v_sb[:, tt, :, 0:64],
                        ps[:, 64:256].rearrange("p (h c) -> p h c", h=3),
                    )

            # replicate each q/k head across the other partition half
            for tq in (t_q0, t_k0, t_q2, t_k2):
                nc.sync.dma_start(out=tq[64:128, :], in_=tq[0:64, :])
            for tq in (t_q1, t_k1):
                nc.sync.dma_start(out=tq[0:64, :], in_=tq[64:128, :])

        # ---- attention + projection ----
        # Pipelined: scores for t-pairs into a 2-bank PSUM tile, one exp ACT
        # per pair, PV accumulation trailing. Each head's normalize +
        # projection tail is deferred into the next head's compute so the PE
        # never drains (keeps the HAM clock un-throttled).
        heads_qk = [(t_k0, t_q0), (t_k1, t_q1), (t_k2, t_q2)]
        with (
            tc.tile_pool(name="psc", bufs=3, space="PSUM") as psc,
            tc.tile_pool(name="pop", bufs=2, space="PSUM") as pop,
        ):

            def make_tail(j, h, po_t, headsT):
                def tail():
                    rc = rcpool.tile([65, 512], f32, tag="rc")
                    nc.vector.reciprocal(rc[64:65, :], po_t[64:65, :])
                    # broadcast 1/S across 64 partitions via K=1 fp32 matmul
                    pb_t = psc.tile([64, 512], f32, tag="ps")
                    nc.tensor.matmul(pb_t, ones[64:65, :], rc[64:65, :])
                    rb = rcpool.tile([64, 512], f32, tag="rb")
                    nc.vector.tensor_copy(rb, pb_t)
                    nc.vector.tensor_mul(headsT[:, h, :], po_t[0:64, :], rb)
                    if h == 2:
                        for e3 in range(3):
                            pp_t = psc.tile([128, 512], f32, tag="ps")
                            for hh in range(3):
                                nc.tensor.matmul(
                                    pp_t,
                                    wpT[:, hh, 128 * e3 : 128 * (e3 + 1)],
                                    headsT[:, hh, :],
                                    start=(hh == 0),
                                    stop=(hh == 2),
                                )
                            y_sb = ypool.tile([128, 512], f32, tag="y_sb")
                            nc.vector.tensor_copy(y_sb, pp_t)
                            nc.sync.dma_start(
                                out=y_d[
                                    128 * e3 : 128 * (e3 + 1),
                                    512 * j : 512 * (j + 1),
                                ],
                                in_=y_sb,
                            )

                return tail

            PV_DELAY = 3  # pairs the PV matmuls trail the exp by

            pending_tail = None
            for j in range(nch):
                headsT = hpool.tile([64, 3, 512], bf16, tag="headsT")
                for h in range(3):
                    kt, qt = heads_qk[h]
                    po_t = pop.tile([65, 512], f32, tag="po")
                    pv_queue = []

                    def flush_pv(po_t=po_t, h=h):
                        t0, e_t = pv_queue.pop(0)
                        for s in range(2):
                            t = t0 + s
                            nc.tensor.matmul(
                                po_t,
                                v_sb[:, t, h, :],
                                e_t[:, s, :],
                                start=(t == 0),
                                stop=(t == nt - 1),
                            )

                    for tp in range(nt // 2):
                        t0 = 2 * tp
                        ps = psc.tile([128, 2, 512], f32, tag="ps")
                        for s in range(2):
                            # row-packed pair: s=0 on partitions 0-63,
                            # s=1 on partitions 64-127 (concurrent on PE)
                            t = t0 + s
                            pb = 64 * s
                            nc.tensor.matmul(
                                ps[:, s, :],
                                kt[pb : pb + 64, 128 * t : 128 * (t + 1)],
                                qt[pb : pb + 64, 512 * j : 512 * (j + 1)],
                            )
                        e_t = epool.tile([128, 2, 512], bf16, tag="e_t")
                        nc.scalar.activation(e_t, ps[:], EXP, scale=SCALE)
                        pv_queue.append((t0, e_t))
                        if len(pv_queue) > PV_DELAY:
                            flush_pv()
                        if tp == 1 and pending_tail is not None:
                            pending_tail()
                            pending_tail = None
                    while pv_queue:
                        flush_pv()
                    pending_tail = make_tail(j, h, po_t, headsT)
            pending_tail()

    nc.compile()
    return nc


def shard_inputs(x, W_qkv, W_proj):
    """Full inputs -> per-core input maps."""
    x = np.ascontiguousarray(np.asarray(x, dtype=np.float32))
    W_qkv = np.ascontiguousarray(np.asarray(W_qkv, dtype=np.float32))
    W_proj = np.ascontiguousarray(np.asarray(W_proj, dtype=np.float32))
    d = W_qkv.shape[1]
    in_maps = []
    for c in range(NCORES):
        b, g = divmod(c, 2)
        sl = slice(GCOLS * g, GCOLS * (g + 1))
        w_g = np.concatenate([W_qkv[0 * d :][sl], W_qkv[1 * d :][sl], W_qkv[2 * d :][sl]], axis=0)
        wp_g = W_proj[:, sl]
        in_maps.append(
            {
                "x": np.ascontiguousarray(x[b]),
                "w_qkv": np.ascontiguousarray(w_g),
                "w_proj": np.ascontiguousarray(wp_g),
            }
        )
    return in_maps


def combine_outputs(yTs, b_proj, n_seq=NSEQ):
    """Per-core yT partials -> full [B, N, D] output."""
    b_proj = np.asarray(b_proj, dtype=np.float32)
    y = np.empty((B, n_seq, D), dtype=np.float32)
    for b in range(B):
        y[b] = (yTs[2 * b] + yTs[2 * b + 1]).T + b_proj
    return y


_NC_CACHE = {}


def kernel(**inputs):
    _ensure_path()
    from concourse.bass_utils import run_bass_kernel_spmd

    x = np.asarray(inputs["x"], dtype=np.float32)
    W_qkv = np.asarray(inputs["W_qkv"], dtype=np.float32)
    W_proj = np.asarray(inputs["W_proj"], dtype=np.float32)
    b_proj = np.asarray(inputs["b_proj"], dtype=np.float32)

    n_seq = x.shape[1]
    if n_seq not in _NC_CACHE:
        _NC_CACHE[n_seq] = build_nc(n_seq)
    nc = _NC_CACHE[n_seq]

    in_maps = shard_inputs(x, W_qkv, W_proj)
    res = run_bass_kernel_spmd(nc, in_maps, core_ids=list(range(NCORES)))
    yTs = [r["yT"] for r in res.results]
    return combine_outputs(yTs, b_proj, n_seq)


if __name__ == "__main__":
    rng = np.random.default_rng(0)
    n = 512
    x = rng.standard_normal((B, n, D), dtype=np.float32)
    wq = (rng.standard_normal((3 * D, D), dtype=np.float32) / np.sqrt(D)).astype(np.float32)
    wp = (rng.standard_normal((D, D), dtype=np.float32) / np.sqrt(D)).astype(np.float32)
    bp = np.zeros(D, np.float32)
    out = kernel(x=x, W_qkv=wq, W_proj=wp, b_proj=bp)
    print(out.shape, out.dtype)


# revision 19
# speedup vs baseline: 1.9557x; 1.0125x over previous
"""Trainium2 Bass kernel for multi-head attention (B=4, N=4096, D=384, H=6).

Sharding: 8 cores = 4 batches x 2 head-groups (3 heads each).
Each core computes, for its (batch, head-group):
    qkv = x @ W_g^T            (W_g = this group's q/k/v rows of W_qkv)
    per head: scoresT = (k q^T)^T laid out [keys(m) partition, queries(n) free]
    e = exp(scoresT * scale)   (no max subtraction; scores ~ N(0,1) after scale)
    PV with an appended ones-column in V: row 64 of the PSUM accumulator is
    the softmax denominator S[n].
    normalized headsT -> partial projection yT_g = Wp_g^T-contracted output
Host sums the two head-group partials per batch, transposes, adds bias.
"""

import os
import sys

import numpy as np

B, NSEQ, D = 4, 4096, 384
H, DH = 6, 64
HPC = 3  # heads per core
NCORES = 8
GCOLS = HPC * DH  # 192
SCALE = float(DH) ** -0.5


def _ensure_path():
    p = "/opt/trn_rl_repo"
    if os.path.isdir(p) and p not in sys.path:
        sys.path.insert(0, p)


def build_nc(n_seq=NSEQ):
    """Build the single-core Bass program (SPMD across 8 cores)."""
    _ensure_path()
    from contextlib import ExitStack

    import concourse.bacc as bacc
    import concourse.tile as tile
    from concourse import mybir
    from concourse.masks import make_identity

    f32 = mybir.dt.float32
    f32r = mybir.dt.float32r
    bf16 = mybir.dt.bfloat16
    EXP = mybir.ActivationFunctionType.Exp

    assert n_seq % 512 == 0
    nt = n_seq // 128
    nch = n_seq // 512

    nc = bacc.Bacc("TRN2", target_bir_lowering=False, debug=False)

    x_d = nc.dram_tensor("x", [n_seq, D], f32, kind="ExternalInput")
    w_d = nc.dram_tensor("w_qkv", [3 * GCOLS, D], f32, kind="ExternalInput")
    wp_d = nc.dram_tensor("w_proj", [D, GCOLS], f32, kind="ExternalInput")
    y_d = nc.dram_tensor("yT", [D, n_seq], f32, kind="ExternalOutput")

    with tile.TileContext(nc) as tc, ExitStack() as ctx:
        constp = ctx.enter_context(tc.tile_pool(name="const", bufs=1))
        persist = ctx.enter_context(tc.tile_pool(name="persist", bufs=1))
        hpool = ctx.enter_context(tc.tile_pool(name="headsp", bufs=2))
        ypool = ctx.enter_context(tc.tile_pool(name="youtp", bufs=3))
        epool = ctx.enter_context(tc.tile_pool(name="epool", bufs=5))
        rcpool = ctx.enter_context(tc.tile_pool(name="rcp", bufs=2))

        ident = constp.tile([128, 128], f32)
        make_identity(nc, ident)
        ones = constp.tile([65, 64], f32)
        nc.vector.memset(ones, 1.0)

        # Transposed weights: wT[p, d3, e] = W_g[e, 128*d3 + p]
        wT = persist.tile([128, 3, 3 * GCOLS], f32r)
        # wpT[p, h, e] = Wp_g[e, 64*h + p]
        wpT = persist.tile([64, 3, D], bf16)
        # Per-head q/k in transposed layout [c, n], bf16, REPLICATED across
        # both 64-partition halves so score matmuls can row-pack in pairs.
        t_q0 = persist.tile([128, n_seq], bf16)
        t_q1 = persist.tile([128, n_seq], bf16)
        t_q2 = persist.tile([128, n_seq], bf16)
        t_k0 = persist.tile([128, n_seq], bf16)
        t_k1 = persist.tile([128, n_seq], bf16)
        t_k2 = persist.tile([128, n_seq], bf16)
        # V in natural layout per n-tile, per head, with a ones column (col 64)
        v_sb = persist.tile([128, nt, 3, 65], bf16)
        ones_nt = constp.tile([128, nt * 3], f32)
        nc.vector.memset(ones_nt, 1.0)
        nc.vector.tensor_copy(
            v_sb[:, :, :, 64:65].rearrange("p t h o -> p (t h o)"), ones_nt
        )

        with (
            tc.tile_pool(name="ldw", bufs=1) as ldw,
            tc.tile_pool(name="ldx", bufs=3) as ldx,
            tc.tile_pool(name="xTp", bufs=1) as xTp,
            tc.tile_pool(name="ptr", bufs=4, space="PSUM") as ptr,
            tc.tile_pool(name="pqk", bufs=2, space="PSUM") as pqk,
            tc.tile_pool(name="pvp", bufs=2, space="PSUM") as pvp,
        ):
            # xT[p, d3, n] = x[n, 128*d3 + p]
            xT = xTp.tile([128, 3, n_seq], f32r)

            # ---- load + transpose weights ----
            for i in range(5):  # 576 rows -> 5 tiles (last is 64)
                rows = min(128, 3 * GCOLS - 128 * i)
                wn = ldw.tile([128, D], f32, tag="wn")
                nc.sync.dma_start(out=wn[:rows, :], in_=w_d[128 * i : 128 * i + rows, :])
                for d3 in range(3):
                    pt = ptr.tile([128, 128], f32, tag="pt")
                    nc.tensor.transpose(
                        pt[:, :rows],
                        wn[:rows, 128 * d3 : 128 * (d3 + 1)],
                        ident[:rows, :rows],
                    )
                    nc.vector.tensor_copy(wT[:, d3, 128 * i : 128 * i + rows], pt[:, :rows])

            for e3 in range(3):
                wpn = ldw.tile([128, GCOLS], f32, tag="wpn")
                nc.sync.dma_start(out=wpn, in_=wp_d[128 * e3 : 128 * (e3 + 1), :])
                for h in range(3):
                    pt = ptr.tile([128, 128], f32, tag="pt")
                    nc.tensor.transpose(pt[:64, :], wpn[:, 64 * h : 64 * (h + 1)], ident)
                    nc.vector.tensor_copy(wpT[:, h, 128 * e3 : 128 * (e3 + 1)], pt[:64, :])

            # ---- load + transpose x, then qkv per chunk ----
            for j in range(nch):
                for tt in range(4 * j, 4 * j + 4):
                    xn = ldx.tile([128, D], f32, tag="xn")
                    nc.sync.dma_start(out=xn, in_=x_d[128 * tt : 128 * (tt + 1), :])
                    pt = ptr.tile([128, 3, 128], f32, tag="pt")
                    for d3 in range(3):
                        nc.tensor.transpose(
                            pt[:, d3, :], xn[:, 128 * d3 : 128 * (d3 + 1)], ident
                        )
                    nc.vector.tensor_copy(xT[:, :, 128 * tt : 128 * (tt + 1)], pt)

                # q/k (transposed layout); each 64-row group is copied to the
                # matching partition half of its replicated destination tile.
                for c0, m, dests in (
                    (0, 128, ((t_q0, 0), (t_q1, 64))),
                    (192, 128, ((t_k0, 0), (t_k1, 64))),
                    (128, 64, ((t_q2, 0),)),
                    (320, 64, ((t_k2, 0),)),
                ):
                    ps = pqk.tile([128, 512], f32, tag="pqk")
                    for d3 in range(3):
                        nc.tensor.matmul(
                            ps[:m, :],
                            wT[:, d3, c0 : c0 + m],
                            xT[:, d3, 512 * j : 512 * (j + 1)],
                            start=(d3 == 0),
                            stop=(d3 == 2),
                        )
                    for dest, p0 in dests:
                        nc.vector.tensor_copy(
                            dest[p0 : p0 + 64, 512 * j : 512 * (j + 1)],
                            ps[p0 : p0 + 64, :],
                        )

                # v in natural layout (rhs widened to 256 cols for full PE rate)
                for tt in range(4 * j, 4 * j + 4):
                    ps = pvp.tile([128, 256], f32, tag="pv")
                    for d3 in range(3):
                        nc.tensor.matmul(
                            ps,
                            xT[:, d3, 128 * tt : 128 * (tt + 1)],
                            wT[:, d3, 320:576],
                            start=(d3 == 0),
                            stop=(d3 == 2),
                        )
                    nc.vector.tensor_copy(
                        v_sb[:, tt, :, 0:64],
                        ps[:, 64:256].rearrange("p (h c) -> p h c", h=3),
                    )

            # replicate each q/k head across the other partition half
            for tq in (t_q0, t_k0, t_q2, t_k2):
                nc.sync.dma_start(out=tq[64:128, :], in_=tq[0:64, :])
            for tq in (t_q1, t_k1):
                nc.sync.dma_start(out=tq[0:64, :], in_=tq[64:128, :])

        # ---- attention + projection ----
        # Pipelined: scores for t-pairs into a 2-bank PSUM tile, one exp ACT
        # per pair, PV accumulation trailing. Each head's normalize +
        # projection tail is deferred into the next head's compute so the PE
        # never drains (keeps the HAM clock un-throttled).
        heads_qk = [(t_k0, t_q0), (t_k1, t_q1), (t_k2, t_q2)]
        with (
            tc.tile_pool(name="psc", bufs=3, space="PSUM") as psc,
            tc.tile_pool(name="pop", bufs=2, space="PSUM") as pop,
        ):

            def make_tail(j, h, po_t, headsT):
                def normalize():
                    rc = rcpool.tile([65, 512], f32, tag="rc")
                    nc.vector.reciprocal(rc[64:65, :], po_t[64:65, :])
                    # broadcast 1/S across 64 partitions via K=1 fp32 matmul
                    pb_t = psc.tile([64, 512], f32, tag="ps")
                    nc.tensor.matmul(pb_t, ones[64:65, :], rc[64:65, :])
                    rb = rcpool.tile([64, 512], f32, tag="rb")
                    nc.vector.tensor_copy(rb, pb_t)
                    nc.vector.tensor_mul(headsT[:, h, :], po_t[0:64, :], rb)

                def proj(e3):
                    def f():
                        pp_t = psc.tile([128, 512], f32, tag="ps")
                        for hh in range(3):
                            nc.tensor.matmul(
                                pp_t,
                                wpT[:, hh, 128 * e3 : 128 * (e3 + 1)],
                                headsT[:, hh, :],
                                start=(hh == 0),
                                stop=(hh == 2),
                            )
                        y_sb = ypool.tile([128, 512], f32, tag="y_sb")
                        nc.vector.tensor_copy(y_sb, pp_t)
                        nc.sync.dma_start(
                            out=y_d[
                                128 * e3 : 128 * (e3 + 1),
                                512 * j : 512 * (j + 1),
                            ],
                            in_=y_sb,
                        )

                    return f

                parts = [normalize]
                if h == 2:
                    parts += [proj(e3) for e3 in range(3)]
                return parts

            PV_DELAY = 3  # pairs the PV matmuls trail the exp by

            pending_tail = []
            for j in range(nch):
                headsT = hpool.tile([64, 3, 512], bf16, tag="headsT")
                for h in range(3):
                    kt, qt = heads_qk[h]
                    po_t = pop.tile([65, 512], f32, tag="po")
                    pv_queue = []

                    def flush_pv(po_t=po_t, h=h):
                        t0, e_t = pv_queue.pop(0)
                        for s in range(2):
                            t = t0 + s
                            nc.tensor.matmul(
                                po_t,
                                v_sb[:, t, h, :],
                                e_t[:, s, :],
                                start=(t == 0),
                                stop=(t == nt - 1),
                            )

                    for tp in range(nt // 2):
                        t0 = 2 * tp
                        ps = psc.tile([128, 2, 512], f32, tag="ps")
                        for s in range(2):
                            # row-packed pair: s=0 on partitions 0-63,
                            # s=1 on partitions 64-127 (concurrent on PE)
                            t = t0 + s
                            pb = 64 * s
                            nc.tensor.matmul(
                                ps[:, s, :],
                                kt[pb : pb + 64, 128 * t : 128 * (t + 1)],
                                qt[pb : pb + 64, 512 * j : 512 * (j + 1)],
                            )
                        e_t = epool.tile([128, 2, 512], bf16, tag="e_t")
                        nc.scalar.activation(e_t, ps[:], EXP, scale=SCALE)
                        pv_queue.append((t0, e_t))
                        if len(pv_queue) > PV_DELAY:
                            flush_pv()
                        if tp >= 1 and pending_tail:
                            pending_tail.pop(0)()
                    while pv_queue:
                        flush_pv()
                    pending_tail = make_tail(j, h, po_t, headsT)
            for part in pending_tail:
                part()

    nc.compile()
    return nc


def shard_inputs(x, W_qkv, W_proj):
    """Full inputs -> per-core input maps."""
    x = np.ascontiguousarray(np.asarray(x, dtype=np.float32))
    W_qkv = np.ascontiguousarray(np.asarray(W_qkv, dtype=np.float32))
    W_proj = np.ascontiguousarray(np.asarray(W_proj, dtype=np.float32))
    d = W_qkv.shape[1]
    in_maps = []
    for c in range(NCORES):
        b, g = divmod(c, 2)
        sl = slice(GCOLS * g, GCOLS * (g + 1))
        w_g = np.concatenate([W_qkv[0 * d :][sl], W_qkv[1 * d :][sl], W_qkv[2 * d :][sl]], axis=0)
        wp_g = W_proj[:, sl]
        in_maps.append(
            {
                "x": np.ascontiguousarray(x[b]),
                "w_qkv": np.ascontiguousarray(w_g),
                "w_proj": np.ascontiguousarray(wp_g),
            }
        )
    return in_maps


def combine_outputs(yTs, b_proj, n_seq=NSEQ):
    """Per-core yT partials -> full [B, N, D] output."""
    b_proj = np.asarray(b_proj, dtype=np.float32)
    y = np.empty((B, n_seq, D), dtype=np.float32)
    for b in range(B):
        y[b] = (yTs[2 * b] + yTs[2 * b + 1]).T + b_proj
    return y


_NC_CACHE = {}


def kernel(**inputs):
    _ensure_path()
    from concourse.bass_utils import run_bass_kernel_spmd

    x = np.asarray(inputs["x"], dtype=np.float32)
    W_qkv = np.asarray(inputs["W_qkv"], dtype=np.float32)
    W_proj = np.asarray(inputs["W_proj"], dtype=np.float32)
    b_proj = np.asarray(inputs["b_proj"], dtype=np.float32)

    n_seq = x.shape[1]
    if n_seq not in _NC_CACHE:
        _NC_CACHE[n_seq] = build_nc(n_seq)
    nc = _NC_CACHE[n_seq]

    in_maps = shard_inputs(x, W_qkv, W_proj)
    res = run_bass_kernel_spmd(nc, in_maps, core_ids=list(range(NCORES)))
    yTs = [r["yT"] for r in res.results]
    return combine_outputs(yTs, b_proj, n_seq)


if __name__ == "__main__":
    rng = np.random.default_rng(0)
    n = 512
    x = rng.standard_normal((B, n, D), dtype=np.float32)
    wq = (rng.standard_normal((3 * D, D), dtype=np.float32) / np.sqrt(D)).astype(np.float32)
    wp = (rng.standard_normal((D, D), dtype=np.float32) / np.sqrt(D)).astype(np.float32)
    bp = np.zeros(D, np.float32)
    out = kernel(x=x, W_qkv=wq, W_proj=wp, b_proj=bp)
    print(out.shape, out.dtype)
